# revision 24
# baseline (speedup 1.0000x reference)
"""Trainium2 Bass kernel for EnhancedGatedFusion (MoE routing, top-2 of 8).

Strategy: data-parallel over tokens across 8 NeuronCores, exploiting top-2
sparsity. The host computes the router (cheap: T*D*E MACs, 0.4% of FLOPs),
picks top-2 experts per token, and pre-gathers tokens into per-expert slot
segments (capacity C, padded; C derived from the actual routing counts).
Each core then runs only the sparse expert compute:

  expert matmuls over gathered slots (bf16, slot-major output, expert bias
  folded into PSUM via a K=1 ones-matmul when nonzero):
      Yge[slot, :] = silu(xg[slot] @ W_e + b_e) * gate[slot]
  each slot row is scattered by a gpsimd indirect DMA into one of two
  token-major stream buffers in DRAM (c1 = every token's top-1 contribution,
  c2 = top-2; padding slots carry an out-of-bounds index and are dropped),
  so the combine is just ct[t] = c1[t] + c2[t] (DVE+gpsimd split).
  ct token-tiles are PE-transposed (bf16) into contraction-major ctT, then
  the dense projection + residual + RMSNorm tail runs token-major (bf16
  proj weights), interleaved with the combine in two token groups so the
  PE pipeline never drains.

This cuts expert FLOPs 8/3x (dense-8 -> top-2 + padding): ~1.06M PE rows
vs 2.4M dense. Measured ~550us/core vs 1220-1456us for the dense baseline.
"""

import sys

for _p in ("/opt/trn_rl_repo",):
    if _p not in sys.path:
        sys.path.insert(0, _p)

from contextlib import ExitStack

import numpy as np

import concourse.bass as bass
import concourse.mybir as mybir
import concourse.tile as tile
from concourse import bacc
from concourse.masks import make_identity

FP32 = mybir.dt.float32
BF16 = mybir.dt.bfloat16
INT32 = mybir.dt.int32
AX = mybir.AxisListType
ALU = mybir.AluOpType
ACTF = mybir.ActivationFunctionType

EPS = 1e-6


def _bcast_ap(ap, nparts=128):
    """Partition-broadcast view of a DRAM AP (step-0 partition dim)."""
    return bass.AP(tensor=ap.tensor, offset=ap.offset, ap=[[0, nparts], *ap.ap])


def build_sparse_moe_nc(D, E, T, C, has_eb=True, has_pb=True, has_nw=True,
                        trn_type="TRN2"):
    """Per-core sparse MoE program. C = per-expert slot capacity (mult of 128).

    has_eb/has_pb/has_nw: emit the expert-bias / proj-bias / norm-weight work
    only when the corresponding tensor is nonzero / non-unit (host-checked;
    the compile cache is keyed on these flags so any input stays correct).
    """
    P = 128
    KO = D // P          # contraction k-tiles (16)
    NTT = T // P         # token tiles (8)
    S = E * C            # total slots
    SPE = C // P         # slot tiles per expert
    WCH = 512            # expert weight moving chunk (psum free dim)
    NWC = D // WCH       # col chunks (4)
    PPW = 512            # proj panel width
    NPP = D // PPW

    nc = bacc.Bacc(trn_type, target_bir_lowering=False, debug=False)

    xgt = nc.dram_tensor("xgt", [D, S], BF16, kind="ExternalInput").ap()
    xr = nc.dram_tensor("xr", [T, D], FP32, kind="ExternalInput").ap()
    # per-slot scatter targets: token row if this slot is the token's
    # stream-1 (top1) / stream-2 (top2) contribution, else T (out of bounds,
    # silently dropped). gs = the token's gate weight for this slot.
    sc1 = nc.dram_tensor("sc1", [S], INT32, kind="ExternalInput").ap()
    sc2 = nc.dram_tensor("sc2", [S], INT32, kind="ExternalInput").ap()
    gs = nc.dram_tensor("gs", [S], FP32, kind="ExternalInput").ap()
    expert_w = nc.dram_tensor("expert_w", [E, D, D], BF16, kind="ExternalInput").ap()
    expert_b = nc.dram_tensor("expert_b", [E, D], BF16, kind="ExternalInput").ap()
    proj_w = nc.dram_tensor("proj_w", [D, D], BF16, kind="ExternalInput").ap()
    proj_b = nc.dram_tensor("proj_b", [D], BF16, kind="ExternalInput").ap()
    norm_w = nc.dram_tensor("norm_w", [D], FP32, kind="ExternalInput").ap()
    out = nc.dram_tensor("out", [T, D], FP32, kind="ExternalOutput").ap()
    c1 = nc.dram_tensor("c1_scratch", [T, D], BF16).ap()
    c2 = nc.dram_tensor("c2_scratch", [T, D], BF16).ap()

    xg_r = xgt.rearrange("(ko p) s -> p ko s", p=P)
    pw_r = proj_w.rearrange("(ko p) c -> p ko c", p=P)
    HKO = KO // 2

    with tile.TileContext(nc) as tc, ExitStack() as ctx:
        v = nc.vector
        s = nc.scalar

        singles = ctx.enter_context(tc.tile_pool(name="singles", bufs=1))
        xg_pool = ctx.enter_context(tc.tile_pool(name="xg_pool", bufs=2))
        w_pool = ctx.enter_context(tc.tile_pool(name="w_pool", bufs=2))
        eb_pool = ctx.enter_context(tc.tile_pool(name="eb_pool", bufs=2))
        sil_pool = ctx.enter_context(tc.tile_pool(name="sil_pool", bufs=3))
        comb_pool = ctx.enter_context(tc.tile_pool(name="comb_pool", bufs=2))
        ct_pool = ctx.enter_context(tc.tile_pool(name="ct_pool", bufs=1))
        y_pool = ctx.enter_context(tc.tile_pool(name="y_pool", bufs=1))
        xres_pool = ctx.enter_context(tc.tile_pool(name="xres_pool", bufs=2))

        # ---- small resident tensors ----
        ones1 = pbsb = nw_rep = None
        if has_eb or has_pb:
            ones1 = singles.tile([1, P], BF16)
            v.memset(ones1, 1.0)
        if has_pb:
            pbsb = singles.tile([1, D], BF16)
            nc.gpsimd.dma_start(out=pbsb, in_=_bcast_ap(proj_b, 1))
        if has_nw:
            nw_rep = singles.tile([P, D], FP32)
            nc.gpsimd.dma_start(out=nw_rep, in_=_bcast_ap(norm_w))
        NST = S // P
        sc1_sb = singles.tile([P, NST], INT32)
        nc.gpsimd.dma_start(out=sc1_sb, in_=sc1.rearrange("(n p) -> p n", p=P))
        sc2_sb = singles.tile([P, NST], INT32)
        nc.gpsimd.dma_start(out=sc2_sb, in_=sc2.rearrange("(n p) -> p n", p=P))
        gs_sb = singles.tile([P, NST], FP32)
        nc.gpsimd.dma_start(out=gs_sb, in_=gs.rearrange("(n p) -> p n", p=P))
        identity = singles.tile([P, P], FP32)
        make_identity(nc, identity)
        eps_t = singles.tile([P, 1], FP32)
        v.memset(eps_t, EPS)

        # ---- expert phase: Yg[slot, :] = silu(xg[slot] @ We + be), slot-major ----
        pse = tc.alloc_tile_pool(name="pse", bufs=6, space="PSUM")
        for e in range(E):
            we_r = expert_w[e].rearrange("(ko p) c -> p ko c", p=P)
            xg_e = xg_pool.tile([P, KO, C], BF16, tag="xg", name=f"xg{e}")
            nc.sync.dma_start(out=xg_e[:, :HKO, :], in_=xg_r[:, :HKO, e * C:(e + 1) * C])
            nc.scalar.dma_start(out=xg_e[:, HKO:, :], in_=xg_r[:, HKO:, e * C:(e + 1) * C])
            if has_eb:
                ebsb = eb_pool.tile([1, D], BF16, tag="eb", name=f"eb{e}")
                nc.scalar.dma_start(out=ebsb, in_=_bcast_ap(expert_b[e], 1))
            yge = sil_pool.tile([P, SPE, D], BF16, tag="yge", bufs=2, name=f"yge{e}")
            for cq in range(NWC):
                wp = w_pool.tile([P, KO, WCH], BF16, tag="wp", name=f"wp{e}_{cq}")
                nc.sync.dma_start(out=wp[:, :HKO, :], in_=we_r[:, :HKO, cq * WCH:(cq + 1) * WCH])
                nc.scalar.dma_start(out=wp[:, HKO:, :], in_=we_r[:, HKO:, cq * WCH:(cq + 1) * WCH])
                for st in range(SPE):
                    stile = e * SPE + st
                    ps = pse.tile([P, WCH], FP32, tag="ps", name=f"ps{e}_{cq}_{st}")
                    if has_eb:
                        # bias via K=1 matmul: ps = ones^T @ eb_chunk
                        nc.tensor.matmul(
                            ps, lhsT=ones1, rhs=ebsb[:, cq * WCH:(cq + 1) * WCH],
                            start=True, stop=False, skip_group_check=True,
                        )
                    for ko in range(KO):
                        nc.tensor.matmul(
                            ps,
                            lhsT=xg_e[:, ko, st * P:(st + 1) * P],
                            rhs=wp[:, ko, :],
                            start=(ko == 0 and not has_eb),
                            stop=(ko == KO - 1),
                            skip_group_check=True,
                        )
                    sg = sil_pool.tile([P, WCH], FP32, tag="sg", name=f"sg{e}_{cq}_{st}")
                    s.activation(sg, ps, ACTF.Sigmoid)
                    # gated silu: (ps * gate_slot) * sigmoid(ps), bf16 out
                    v.scalar_tensor_tensor(
                        out=yge[:, st, cq * WCH:(cq + 1) * WCH],
                        in0=ps, scalar=gs_sb[:, stile:stile + 1], in1=sg,
                        op0=ALU.mult, op1=ALU.mult,
                    )
            for st in range(SPE):
                stile = e * SPE + st
                nc.gpsimd.indirect_dma_start(
                    out=c1, out_offset=bass.IndirectOffsetOnAxis(
                        ap=sc1_sb[:, stile:stile + 1], axis=0),
                    in_=yge[:, st, :], in_offset=None,
                    bounds_check=T - 1, oob_is_err=False,
                )
                nc.gpsimd.indirect_dma_start(
                    out=c2, out_offset=bass.IndirectOffsetOnAxis(
                        ap=sc2_sb[:, stile:stile + 1], axis=0),
                    in_=yge[:, st, :], in_offset=None,
                    bounds_check=T - 1, oob_is_err=False,
                )
        pse.release()

        # ---- combine + projection, interleaved in two token groups ----
        # combine: ct[t] = g1*Yg[s1[t]] + g2*Yg[s2[t]] (row gather via gpsimd
        # indirect DMA), PE-transposed into contraction-major ctT (bf16).
        # proj group tg only needs ctT token tiles of that group, so the PE
        # stream is: transposes(g0) -> proj(g0) -> transposes(g1) -> proj(g1),
        # letting group-1 gathers/DVE run under group-0 proj matmuls.
        ctT = ct_pool.tile([P, KO, T], BF16)
        id_bf = singles.tile([P, P], BF16)
        v.tensor_copy(out=id_bf, in_=identity)
        NG = 2
        TG = NTT // NG
        KOC = 4                     # ko per psum->sbuf copy batch
        with (
            tc.tile_pool(name="psT", bufs=2, space="PSUM") as psT,
            tc.tile_pool(name="psp", bufs=6, space="PSUM") as psp,
            tc.tile_pool(name="nsm", bufs=2) as nsm,
        ):
            HD = D // 2

            def combine_tt(tt):
                y1 = comb_pool.tile([P, D], BF16, tag="y1", name=f"y1_{tt}")
                nc.sync.dma_start(out=y1, in_=c1[tt * P:(tt + 1) * P, :])
                y2 = comb_pool.tile([P, D], BF16, tag="y2", name=f"y2_{tt}")
                nc.scalar.dma_start(out=y2, in_=c2[tt * P:(tt + 1) * P, :])
                ctt = comb_pool.tile([P, D], BF16, tag="ctt", name=f"ctt{tt}")
                # gpsimd tensor ops are ~2.6x slower than DVE: give it 1/4
                GPW = D // 4
                for csl, eng in ((slice(0, D - GPW), v),
                                 (slice(D - GPW, D), nc.gpsimd)):
                    eng.tensor_tensor(
                        out=ctt[:, csl], in0=y1[:, csl], in1=y2[:, csl],
                        op=ALU.add,
                    )
                for kb in range(KO // KOC):
                    pst = psT.tile([P, KOC * P], BF16, tag="pst",
                                   name=f"pst{tt}_{kb}")
                    for kk in range(KOC):
                        ko = kb * KOC + kk
                        nc.tensor.transpose(
                            pst[:, kk * P:(kk + 1) * P],
                            ctt[:, ko * P:(ko + 1) * P], id_bf,
                        )
                    dst = ctT[:, kb * KOC:(kb + 1) * KOC, tt * P:(tt + 1) * P]
                    src = pst.rearrange("p (k c) -> p k c", k=KOC)
                    if kb % 2 == 0:
                        v.tensor_copy(out=dst, in_=src)
                    else:
                        s.activation(dst, src, ACTF.Copy)

            def emit_norm(y_all, tt_local, tt):
                y_t = y_all[:, tt_local, :]
                sq = nsm.tile([P, HD], FP32, tag="sq", bufs=1, name=f"sq{tt}")
                ssa = nsm.tile([P, 1], FP32, tag="ssa", name=f"ssa{tt}")
                ssb = nsm.tile([P, 1], FP32, tag="ssb", name=f"ssb{tt}")
                s.activation(sq, y_t[:, :HD], ACTF.Square, accum_out=ssa)
                s.activation(sq, y_t[:, HD:], ACTF.Square, accum_out=ssb)
                ssum = nsm.tile([P, 1], FP32, tag="ssum", name=f"ssum{tt}")
                v.tensor_tensor(out=ssum, in0=ssa, in1=ssb, op=ALU.add)
                rms = nsm.tile([P, 1], FP32, tag="rms", name=f"rms{tt}")
                s.activation(rms, ssum, ACTF.Sqrt, bias=eps_t, scale=1.0 / D)
                rinv = nsm.tile([P, 1], FP32, tag="rinv", name=f"rinv{tt}")
                v.reciprocal(rinv, rms)
                s.mul(y_t, y_t, rinv)
                if has_nw:
                    v.tensor_tensor(out=y_t, in0=y_t, in1=nw_rep, op=ALU.mult)
                oeng = nc.sync if tt % 2 == 0 else nc.scalar
                oeng.dma_start(out=out[tt * P:(tt + 1) * P, :], in_=y_t)

            for tg in range(NG):
                for tt in range(tg * TG, (tg + 1) * TG):
                    combine_tt(tt)
                y_all = y_pool.tile([P, TG, D], FP32, tag="y", name=f"y_all{tg}")
                for pp in range(NPP):
                    pwp = w_pool.tile([P, KO, PPW], BF16, tag="wp", name=f"pwp{tg}_{pp}")
                    nc.sync.dma_start(out=pwp[:, :HKO, :], in_=pw_r[:, :HKO, pp * PPW:(pp + 1) * PPW])
                    nc.scalar.dma_start(out=pwp[:, HKO:, :], in_=pw_r[:, HKO:, pp * PPW:(pp + 1) * PPW])
                    for tt_local in range(TG):
                        tt = tg * TG + tt_local
                        ps_o = psp.tile([P, PPW], FP32, tag="ps", name=f"pso{pp}_{tt}")
                        if has_pb:
                            nc.tensor.matmul(
                                ps_o, lhsT=ones1, rhs=pbsb[:, pp * PPW:(pp + 1) * PPW],
                                start=True, stop=False, skip_group_check=True,
                            )
                        for ko in range(KO):
                            nc.tensor.matmul(
                                ps_o,
                                lhsT=ctT[:, ko, tt * P:(tt + 1) * P],
                                rhs=pwp[:, ko, :],
                                start=(ko == 0 and not has_pb),
                                stop=(ko == KO - 1),
                                skip_group_check=True,
                            )
                        xres = xres_pool.tile([P, PPW], FP32, tag="xres", name=f"xres{pp}_{tt}")
                        nc.scalar.dma_start(
                            out=xres, in_=xr[tt * P:(tt + 1) * P, pp * PPW:(pp + 1) * PPW]
                        )
                        y_sl = y_all[:, tt_local, pp * PPW:(pp + 1) * PPW]
                        v.tensor_tensor(out=y_sl, in0=ps_o, in1=xres, op=ALU.add)
                        if pp == NPP - 1:
                            emit_norm(y_all, tt_local, tt)

    nc.compile()
    return nc


# ---- full-problem entry point ----
_B, _SEQ, _D, _E = 4, 2048, 2048, 8
_NCORES = 8
_T = _B * _SEQ // _NCORES

_nc_cache = {}


def _get_nc(key):
    if key not in _nc_cache:
        C, has_eb, has_pb, has_nw = key
        _nc_cache[key] = build_sparse_moe_nc(_D, _E, _T, C, has_eb=has_eb,
                                             has_pb=has_pb, has_nw=has_nw)
    return _nc_cache[key]


def _route(xf, router_w, router_b):
    """Host router: fp32 logits (matching reference precision), top-2, gates."""
    logits = xf @ router_w + router_b                       # [Tn, E] fp32
    order = np.argsort(-logits, axis=1, kind="stable")      # jax top_k tie-break
    i1 = order[:, 0]
    i2 = order[:, 1]
    tn = np.arange(logits.shape[0])
    l1 = logits[tn, i1].astype(np.float64)
    l2 = logits[tn, i2].astype(np.float64)
    e2 = np.exp(l2 - l1)
    den = 1.0 + e2
    w1 = (1.0 / den).astype(np.float32)
    w2 = (e2 / den).astype(np.float32)
    return i1, i2, w1, w2


def _make_in_maps(xf, router_w, router_b, expert_w, expert_b, proj_w, proj_b,
                  norm_w):
    import ml_dtypes

    i1, i2, w1, w2 = _route(xf, router_w, router_b)

    counts = np.zeros((_NCORES, _E), np.int64)
    for c in range(_NCORES):
        sl = slice(c * _T, (c + 1) * _T)
        for e in range(_E):
            counts[c, e] = ((i1[sl] == e) | (i2[sl] == e)).sum()
    C = max(128, int(np.ceil(counts.max() / 128)) * 128)
    S = _E * C
    key = (C, bool(np.any(expert_b)), bool(np.any(proj_b)),
           bool(np.any(norm_w != 1.0)))

    ew_b = np.ascontiguousarray(expert_w.astype(ml_dtypes.bfloat16))
    eb_b = np.ascontiguousarray(expert_b.astype(ml_dtypes.bfloat16))
    pw_b = np.ascontiguousarray(proj_w.astype(ml_dtypes.bfloat16))
    pb_b = np.ascontiguousarray(proj_b.astype(ml_dtypes.bfloat16))
    nw_f = np.ascontiguousarray(norm_w.astype(np.float32))

    in_maps = []
    for c in range(_NCORES):
        sl = slice(c * _T, (c + 1) * _T)
        xc = xf[sl]
        i1c, i2c, w1c, w2c = i1[sl], i2[sl], w1[sl], w2[sl]
        slot_tokens = np.zeros(S, np.int64)   # pad slots point at token 0 (unused)
        sc1c = np.full(S, _T, np.int32)       # T = out-of-bounds sentinel
        sc2c = np.full(S, _T, np.int32)
        gsc = np.zeros(S, np.float32)
        for e in range(_E):
            toks = np.nonzero((i1c == e) | (i2c == e))[0]
            base = e * C
            slot_tokens[base:base + len(toks)] = toks
            slots = base + np.arange(len(toks))
            is1 = i1c[toks] == e
            sc1c[slots[is1]] = toks[is1]
            sc2c[slots[~is1]] = toks[~is1]
            gsc[slots[is1]] = w1c[toks[is1]]
            gsc[slots[~is1]] = w2c[toks[~is1]]
        xg = xc[slot_tokens]                                  # [S, D] fp32
        xgt = np.ascontiguousarray(xg.T.astype(ml_dtypes.bfloat16))  # [D, S]
        in_maps.append({
            "xgt": xgt,
            "xr": np.ascontiguousarray(xc),
            "sc1": sc1c,
            "sc2": sc2c,
            "gs": gsc,
            "expert_w": ew_b,
            "expert_b": eb_b,
            "proj_w": pw_b,
            "proj_b": pb_b,
            "norm_w": nw_f,
        })
    return in_maps, key


def kernel(x, router_w, router_b, expert_w, expert_b, proj_w, proj_b, norm_w):
    from concourse import bass_utils

    x = np.asarray(x, np.float32)
    router_w = np.asarray(router_w, np.float32)
    router_b = np.asarray(router_b, np.float32)
    expert_w = np.asarray(expert_w, np.float32)
    expert_b = np.asarray(expert_b, np.float32)
    proj_w = np.asarray(proj_w, np.float32)
    proj_b = np.asarray(proj_b, np.float32)
    norm_w = np.asarray(norm_w, np.float32)

    xf = x.reshape(-1, _D)
    in_maps, key = _make_in_maps(xf, router_w, router_b, expert_w, expert_b,
                                 proj_w, proj_b, norm_w)
    nc = _get_nc(key)
    res = bass_utils.run_bass_kernel_spmd(nc, in_maps, core_ids=list(range(_NCORES)))
    outs = [res.results[c]["out"] for c in range(_NCORES)]
    return np.concatenate(outs, axis=0).reshape(_B, _SEQ, _D).astype(np.float32)


# revision 25
# speedup vs baseline: 1.0032x; 1.0032x over previous
"""Trainium2 Bass kernel for EnhancedGatedFusion (MoE routing, top-2 of 8).

Strategy: data-parallel over tokens across 8 NeuronCores, exploiting top-2
sparsity. The host computes the router (cheap: T*D*E MACs, 0.4% of FLOPs),
picks top-2 experts per token, and pre-gathers tokens into per-expert slot
segments (capacity C, padded; C derived from the actual routing counts).
Each core then runs only the sparse expert compute:

  expert matmuls over gathered slots (bf16, slot-major output, expert bias
  folded into PSUM via a K=1 ones-matmul when nonzero):
      Yge[slot, :] = silu(xg[slot] @ W_e + b_e) * gate[slot]
  each slot row is scattered by a gpsimd indirect DMA into one of two
  token-major stream buffers in DRAM (c1 = every token's top-1 contribution,
  c2 = top-2; padding slots carry an out-of-bounds index and are dropped),
  so the combine is just ct[t] = c1[t] + c2[t] (DVE+gpsimd split).
  ct token-tiles are PE-transposed (bf16) into contraction-major ctT, then
  the dense projection + residual + RMSNorm tail runs token-major (bf16
  proj weights), interleaved with the combine in two token groups so the
  PE pipeline never drains.

This cuts expert FLOPs 8/3x (dense-8 -> top-2 + padding): ~1.06M PE rows
vs 2.4M dense. Measured ~550us/core vs 1220-1456us for the dense baseline.
"""

import sys

for _p in ("/opt/trn_rl_repo",):
    if _p not in sys.path:
        sys.path.insert(0, _p)

from contextlib import ExitStack

import numpy as np

import concourse.bass as bass
import concourse.mybir as mybir
import concourse.tile as tile
from concourse import bacc
from concourse.masks import make_identity

FP32 = mybir.dt.float32
BF16 = mybir.dt.bfloat16
INT32 = mybir.dt.int32
AX = mybir.AxisListType
ALU = mybir.AluOpType
ACTF = mybir.ActivationFunctionType

EPS = 1e-6


def _bcast_ap(ap, nparts=128):
    """Partition-broadcast view of a DRAM AP (step-0 partition dim)."""
    return bass.AP(tensor=ap.tensor, offset=ap.offset, ap=[[0, nparts], *ap.ap])


def build_sparse_moe_nc(D, E, T, C, has_eb=True, has_pb=True, has_nw=True,
                        trn_type="TRN2"):
    """Per-core sparse MoE program. C = per-expert slot capacity (mult of 128).

    has_eb/has_pb/has_nw: emit the expert-bias / proj-bias / norm-weight work
    only when the corresponding tensor is nonzero / non-unit (host-checked;
    the compile cache is keyed on these flags so any input stays correct).
    """
    P = 128
    KO = D // P          # contraction k-tiles (16)
    NTT = T // P         # token tiles (8)
    S = E * C            # total slots
    SPE = C // P         # slot tiles per expert
    WCH = 512            # expert weight moving chunk (psum free dim)
    NWC = D // WCH       # col chunks (4)
    PPW = 512            # proj panel width
    NPP = D // PPW

    nc = bacc.Bacc(trn_type, target_bir_lowering=False, debug=False)

    xgt = nc.dram_tensor("xgt", [D, S], BF16, kind="ExternalInput").ap()
    xr = nc.dram_tensor("xr", [T, D], BF16, kind="ExternalInput").ap()
    # per-slot scatter targets: token row if this slot is the token's
    # stream-1 (top1) / stream-2 (top2) contribution, else T (out of bounds,
    # silently dropped). gs = the token's gate weight for this slot.
    sc1 = nc.dram_tensor("sc1", [S], INT32, kind="ExternalInput").ap()
    sc2 = nc.dram_tensor("sc2", [S], INT32, kind="ExternalInput").ap()
    gs = nc.dram_tensor("gs", [S], FP32, kind="ExternalInput").ap()
    expert_w = nc.dram_tensor("expert_w", [E, D, D], BF16, kind="ExternalInput").ap()
    expert_b = nc.dram_tensor("expert_b", [E, D], BF16, kind="ExternalInput").ap()
    proj_w = nc.dram_tensor("proj_w", [D, D], BF16, kind="ExternalInput").ap()
    proj_b = nc.dram_tensor("proj_b", [D], BF16, kind="ExternalInput").ap()
    norm_w = nc.dram_tensor("norm_w", [D], FP32, kind="ExternalInput").ap()
    out = nc.dram_tensor("out", [T, D], BF16, kind="ExternalOutput").ap()
    c1 = nc.dram_tensor("c1_scratch", [T, D], BF16).ap()
    c2 = nc.dram_tensor("c2_scratch", [T, D], BF16).ap()

    xg_r = xgt.rearrange("(ko p) s -> p ko s", p=P)
    pw_r = proj_w.rearrange("(ko p) c -> p ko c", p=P)
    HKO = KO // 2

    with tile.TileContext(nc) as tc, ExitStack() as ctx:
        v = nc.vector
        s = nc.scalar

        singles = ctx.enter_context(tc.tile_pool(name="singles", bufs=1))
        xg_pool = ctx.enter_context(tc.tile_pool(name="xg_pool", bufs=2))
        w_pool = ctx.enter_context(tc.tile_pool(name="w_pool", bufs=2))
        eb_pool = ctx.enter_context(tc.tile_pool(name="eb_pool", bufs=2))
        sil_pool = ctx.enter_context(tc.tile_pool(name="sil_pool", bufs=3))
        comb_pool = ctx.enter_context(tc.tile_pool(name="comb_pool", bufs=2))
        ct_pool = ctx.enter_context(tc.tile_pool(name="ct_pool", bufs=1))
        y_pool = ctx.enter_context(tc.tile_pool(name="y_pool", bufs=1))
        xres_pool = ctx.enter_context(tc.tile_pool(name="xres_pool", bufs=2))

        # ---- small resident tensors ----
        ones1 = pbsb = nw_rep = None
        if has_eb or has_pb:
            ones1 = singles.tile([1, P], BF16)
            v.memset(ones1, 1.0)
        if has_pb:
            pbsb = singles.tile([1, D], BF16)
            nc.gpsimd.dma_start(out=pbsb, in_=_bcast_ap(proj_b, 1))
        if has_nw:
            nw_rep = singles.tile([P, D], FP32)
            nc.gpsimd.dma_start(out=nw_rep, in_=_bcast_ap(norm_w))
        NST = S // P
        sc1_sb = singles.tile([P, NST], INT32)
        nc.gpsimd.dma_start(out=sc1_sb, in_=sc1.rearrange("(n p) -> p n", p=P))
        sc2_sb = singles.tile([P, NST], INT32)
        nc.gpsimd.dma_start(out=sc2_sb, in_=sc2.rearrange("(n p) -> p n", p=P))
        gs_sb = singles.tile([P, NST], FP32)
        nc.gpsimd.dma_start(out=gs_sb, in_=gs.rearrange("(n p) -> p n", p=P))
        identity = singles.tile([P, P], FP32)
        make_identity(nc, identity)
        eps_t = singles.tile([P, 1], FP32)
        v.memset(eps_t, EPS)

        # ---- expert phase: Yg[slot, :] = silu(xg[slot] @ We + be), slot-major ----
        pse = tc.alloc_tile_pool(name="pse", bufs=6, space="PSUM")
        for e in range(E):
            we_r = expert_w[e].rearrange("(ko p) c -> p ko c", p=P)
            xg_e = xg_pool.tile([P, KO, C], BF16, tag="xg", name=f"xg{e}")
            nc.sync.dma_start(out=xg_e[:, :HKO, :], in_=xg_r[:, :HKO, e * C:(e + 1) * C])
            nc.scalar.dma_start(out=xg_e[:, HKO:, :], in_=xg_r[:, HKO:, e * C:(e + 1) * C])
            if has_eb:
                ebsb = eb_pool.tile([1, D], BF16, tag="eb", name=f"eb{e}")
                nc.scalar.dma_start(out=ebsb, in_=_bcast_ap(expert_b[e], 1))
            yge = sil_pool.tile([P, SPE, D], BF16, tag="yge", bufs=2, name=f"yge{e}")
            for cq in range(NWC):
                wp = w_pool.tile([P, KO, WCH], BF16, tag="wp", name=f"wp{e}_{cq}")
                nc.sync.dma_start(out=wp[:, :HKO, :], in_=we_r[:, :HKO, cq * WCH:(cq + 1) * WCH])
                nc.scalar.dma_start(out=wp[:, HKO:, :], in_=we_r[:, HKO:, cq * WCH:(cq + 1) * WCH])
                for st in range(SPE):
                    stile = e * SPE + st
                    ps = pse.tile([P, WCH], FP32, tag="ps", name=f"ps{e}_{cq}_{st}")
                    if has_eb:
                        # bias via K=1 matmul: ps = ones^T @ eb_chunk
                        nc.tensor.matmul(
                            ps, lhsT=ones1, rhs=ebsb[:, cq * WCH:(cq + 1) * WCH],
                            start=True, stop=False, skip_group_check=True,
                        )
                    for ko in range(KO):
                        nc.tensor.matmul(
                            ps,
                            lhsT=xg_e[:, ko, st * P:(st + 1) * P],
                            rhs=wp[:, ko, :],
                            start=(ko == 0 and not has_eb),
                            stop=(ko == KO - 1),
                            skip_group_check=True,
                        )
                    sg = sil_pool.tile([P, WCH], FP32, tag="sg", name=f"sg{e}_{cq}_{st}")
                    s.activation(sg, ps, ACTF.Sigmoid)
                    # gated silu: (ps * gate_slot) * sigmoid(ps), bf16 out
                    v.scalar_tensor_tensor(
                        out=yge[:, st, cq * WCH:(cq + 1) * WCH],
                        in0=ps, scalar=gs_sb[:, stile:stile + 1], in1=sg,
                        op0=ALU.mult, op1=ALU.mult,
                    )
                    if cq == NWC - 1:
                        # this slot-tile's row is complete: scatter it now so
                        # the gpsimd queue drains before the expert loop ends
                        nc.gpsimd.indirect_dma_start(
                            out=c1, out_offset=bass.IndirectOffsetOnAxis(
                                ap=sc1_sb[:, stile:stile + 1], axis=0),
                            in_=yge[:, st, :], in_offset=None,
                            bounds_check=T - 1, oob_is_err=False,
                        )
                        nc.gpsimd.indirect_dma_start(
                            out=c2, out_offset=bass.IndirectOffsetOnAxis(
                                ap=sc2_sb[:, stile:stile + 1], axis=0),
                            in_=yge[:, st, :], in_offset=None,
                            bounds_check=T - 1, oob_is_err=False,
                        )
        pse.release()

        # ---- combine + projection, interleaved in two token groups ----
        # combine: ct[t] = g1*Yg[s1[t]] + g2*Yg[s2[t]] (row gather via gpsimd
        # indirect DMA), PE-transposed into contraction-major ctT (bf16).
        # proj group tg only needs ctT token tiles of that group, so the PE
        # stream is: transposes(g0) -> proj(g0) -> transposes(g1) -> proj(g1),
        # letting group-1 gathers/DVE run under group-0 proj matmuls.
        ctT = ct_pool.tile([P, KO, T], BF16)
        id_bf = singles.tile([P, P], BF16)
        v.tensor_copy(out=id_bf, in_=identity)
        NG = 2
        TG = NTT // NG
        KOC = 4                     # ko per psum->sbuf copy batch
        with (
            tc.tile_pool(name="psT", bufs=2, space="PSUM") as psT,
            tc.tile_pool(name="psp", bufs=6, space="PSUM") as psp,
            tc.tile_pool(name="nsm", bufs=2) as nsm,
        ):
            HD = D // 2

            def combine_tt(tt):
                y1 = comb_pool.tile([P, D], BF16, tag="y1", name=f"y1_{tt}")
                nc.sync.dma_start(out=y1, in_=c1[tt * P:(tt + 1) * P, :])
                y2 = comb_pool.tile([P, D], BF16, tag="y2", name=f"y2_{tt}")
                nc.scalar.dma_start(out=y2, in_=c2[tt * P:(tt + 1) * P, :])
                ctt = comb_pool.tile([P, D], BF16, tag="ctt", name=f"ctt{tt}")
                # gpsimd tensor ops are ~2.6x slower than DVE: give it 1/4
                GPW = D // 4
                for csl, eng in ((slice(0, D - GPW), v),
                                 (slice(D - GPW, D), nc.gpsimd)):
                    eng.tensor_tensor(
                        out=ctt[:, csl], in0=y1[:, csl], in1=y2[:, csl],
                        op=ALU.add,
                    )
                for kb in range(KO // KOC):
                    pst = psT.tile([P, KOC * P], BF16, tag="pst",
                                   name=f"pst{tt}_{kb}")
                    for kk in range(KOC):
                        ko = kb * KOC + kk
                        nc.tensor.transpose(
                            pst[:, kk * P:(kk + 1) * P],
                            ctt[:, ko * P:(ko + 1) * P], id_bf,
                        )
                    dst = ctT[:, kb * KOC:(kb + 1) * KOC, tt * P:(tt + 1) * P]
                    src = pst.rearrange("p (k c) -> p k c", k=KOC)
                    if kb % 2 == 0:
                        v.tensor_copy(out=dst, in_=src)
                    else:
                        s.activation(dst, src, ACTF.Copy)

            def emit_norm(y_all, tt_local, tt):
                y_t = y_all[:, tt_local, :]
                sq = nsm.tile([P, HD], FP32, tag="sq", bufs=1, name=f"sq{tt}")
                ssa = nsm.tile([P, 1], FP32, tag="ssa", name=f"ssa{tt}")
                ssb = nsm.tile([P, 1], FP32, tag="ssb", name=f"ssb{tt}")
                s.activation(sq, y_t[:, :HD], ACTF.Square, accum_out=ssa)
                s.activation(sq, y_t[:, HD:], ACTF.Square, accum_out=ssb)
                ssum = nsm.tile([P, 1], FP32, tag="ssum", name=f"ssum{tt}")
                v.tensor_tensor(out=ssum, in0=ssa, in1=ssb, op=ALU.add)
                rms = nsm.tile([P, 1], FP32, tag="rms", name=f"rms{tt}")
                s.activation(rms, ssum, ACTF.Sqrt, bias=eps_t, scale=1.0 / D)
                rinv = nsm.tile([P, 1], FP32, tag="rinv", name=f"rinv{tt}")
                v.reciprocal(rinv, rms)
                y_b = comb_pool.tile([P, D], BF16, tag="y1", name=f"yb{tt}")
                if has_nw:
                    s.mul(y_t, y_t, rinv)
                    v.tensor_tensor(out=y_b, in0=y_t, in1=nw_rep, op=ALU.mult)
                else:
                    s.mul(y_b, y_t, rinv)
                oeng = nc.sync if tt % 2 == 0 else nc.scalar
                oeng.dma_start(out=out[tt * P:(tt + 1) * P, :], in_=y_b)

            for tg in range(NG):
                for tt in range(tg * TG, (tg + 1) * TG):
                    combine_tt(tt)
                y_all = y_pool.tile([P, TG, D], FP32, tag="y", name=f"y_all{tg}")
                for pp in range(NPP):
                    pwp = w_pool.tile([P, KO, PPW], BF16, tag="wp", name=f"pwp{tg}_{pp}")
                    nc.sync.dma_start(out=pwp[:, :HKO, :], in_=pw_r[:, :HKO, pp * PPW:(pp + 1) * PPW])
                    nc.scalar.dma_start(out=pwp[:, HKO:, :], in_=pw_r[:, HKO:, pp * PPW:(pp + 1) * PPW])
                    for tt_local in range(TG):
                        tt = tg * TG + tt_local
                        ps_o = psp.tile([P, PPW], FP32, tag="ps", name=f"pso{pp}_{tt}")
                        if has_pb:
                            nc.tensor.matmul(
                                ps_o, lhsT=ones1, rhs=pbsb[:, pp * PPW:(pp + 1) * PPW],
                                start=True, stop=False, skip_group_check=True,
                            )
                        for ko in range(KO):
                            nc.tensor.matmul(
                                ps_o,
                                lhsT=ctT[:, ko, tt * P:(tt + 1) * P],
                                rhs=pwp[:, ko, :],
                                start=(ko == 0 and not has_pb),
                                stop=(ko == KO - 1),
                                skip_group_check=True,
                            )
                        xres = xres_pool.tile([P, PPW], BF16, tag="xres", name=f"xres{pp}_{tt}")
                        nc.scalar.dma_start(
                            out=xres, in_=xr[tt * P:(tt + 1) * P, pp * PPW:(pp + 1) * PPW]
                        )
                        y_sl = y_all[:, tt_local, pp * PPW:(pp + 1) * PPW]
                        v.tensor_tensor(out=y_sl, in0=ps_o, in1=xres, op=ALU.add)
                        if pp == NPP - 1:
                            emit_norm(y_all, tt_local, tt)

    nc.compile()
    return nc


# ---- full-problem entry point ----
_B, _SEQ, _D, _E = 4, 2048, 2048, 8
_NCORES = 8
_T = _B * _SEQ // _NCORES

_nc_cache = {}


def _get_nc(key):
    if key not in _nc_cache:
        C, has_eb, has_pb, has_nw = key
        _nc_cache[key] = build_sparse_moe_nc(_D, _E, _T, C, has_eb=has_eb,
                                             has_pb=has_pb, has_nw=has_nw)
    return _nc_cache[key]


def _route(xf, router_w, router_b):
    """Host router: fp32 logits (matching reference precision), top-2, gates."""
    logits = xf @ router_w + router_b                       # [Tn, E] fp32
    order = np.argsort(-logits, axis=1, kind="stable")      # jax top_k tie-break
    i1 = order[:, 0]
    i2 = order[:, 1]
    tn = np.arange(logits.shape[0])
    l1 = logits[tn, i1].astype(np.float64)
    l2 = logits[tn, i2].astype(np.float64)
    e2 = np.exp(l2 - l1)
    den = 1.0 + e2
    w1 = (1.0 / den).astype(np.float32)
    w2 = (e2 / den).astype(np.float32)
    return i1, i2, w1, w2


def _make_in_maps(xf, router_w, router_b, expert_w, expert_b, proj_w, proj_b,
                  norm_w):
    import ml_dtypes

    i1, i2, w1, w2 = _route(xf, router_w, router_b)

    counts = np.zeros((_NCORES, _E), np.int64)
    for c in range(_NCORES):
        sl = slice(c * _T, (c + 1) * _T)
        for e in range(_E):
            counts[c, e] = ((i1[sl] == e) | (i2[sl] == e)).sum()
    C = max(128, int(np.ceil(counts.max() / 128)) * 128)
    S = _E * C
    key = (C, bool(np.any(expert_b)), bool(np.any(proj_b)),
           bool(np.any(norm_w != 1.0)))

    ew_b = np.ascontiguousarray(expert_w.astype(ml_dtypes.bfloat16))
    eb_b = np.ascontiguousarray(expert_b.astype(ml_dtypes.bfloat16))
    pw_b = np.ascontiguousarray(proj_w.astype(ml_dtypes.bfloat16))
    pb_b = np.ascontiguousarray(proj_b.astype(ml_dtypes.bfloat16))
    nw_f = np.ascontiguousarray(norm_w.astype(np.float32))

    in_maps = []
    for c in range(_NCORES):
        sl = slice(c * _T, (c + 1) * _T)
        xc = xf[sl]
        i1c, i2c, w1c, w2c = i1[sl], i2[sl], w1[sl], w2[sl]
        slot_tokens = np.zeros(S, np.int64)   # pad slots point at token 0 (unused)
        sc1c = np.full(S, _T, np.int32)       # T = out-of-bounds sentinel
        sc2c = np.full(S, _T, np.int32)
        gsc = np.zeros(S, np.float32)
        for e in range(_E):
            toks = np.nonzero((i1c == e) | (i2c == e))[0]
            base = e * C
            slot_tokens[base:base + len(toks)] = toks
            slots = base + np.arange(len(toks))
            is1 = i1c[toks] == e
            sc1c[slots[is1]] = toks[is1]
            sc2c[slots[~is1]] = toks[~is1]
            gsc[slots[is1]] = w1c[toks[is1]]
            gsc[slots[~is1]] = w2c[toks[~is1]]
        xg = xc[slot_tokens]                                  # [S, D] fp32
        xgt = np.ascontiguousarray(xg.T.astype(ml_dtypes.bfloat16))  # [D, S]
        in_maps.append({
            "xgt": xgt,
            "xr": np.ascontiguousarray(xc.astype(ml_dtypes.bfloat16)),
            "sc1": sc1c,
            "sc2": sc2c,
            "gs": gsc,
            "expert_w": ew_b,
            "expert_b": eb_b,
            "proj_w": pw_b,
            "proj_b": pb_b,
            "norm_w": nw_f,
        })
    return in_maps, key


def kernel(x, router_w, router_b, expert_w, expert_b, proj_w, proj_b, norm_w):
    from concourse import bass_utils

    x = np.asarray(x, np.float32)
    router_w = np.asarray(router_w, np.float32)
    router_b = np.asarray(router_b, np.float32)
    expert_w = np.asarray(expert_w, np.float32)
    expert_b = np.asarray(expert_b, np.float32)
    proj_w = np.asarray(proj_w, np.float32)
    proj_b = np.asarray(proj_b, np.float32)
    norm_w = np.asarray(norm_w, np.float32)

    xf = x.reshape(-1, _D)
    in_maps, key = _make_in_maps(xf, router_w, router_b, expert_w, expert_b,
                                 proj_w, proj_b, norm_w)
    nc = _get_nc(key)
    res = bass_utils.run_bass_kernel_spmd(nc, in_maps, core_ids=list(range(_NCORES)))
    outs = [res.results[c]["out"] for c in range(_NCORES)]
    return np.concatenate(outs, axis=0).reshape(_B, _SEQ, _D).astype(np.float32)
